# revision 6
# baseline (speedup 1.0000x reference)
"""Trainium2 Bass kernel v6: v-stationary col-packed PV in u-space.

v5 bottleneck: PV ran e-stationary (lhsT = exp(S^T) [128j x 128i] chunk,
rhs = vw [128, 65]) -> 1024 matmuls, each LoadStationary-bound (~80ns
measured: LS of a 128-col stationary ~53ns > 27ns of moving stream) ->
~82us of PE time on PV alone.

v6 flips PV: stationary = v_aug^T chunk [128 j, 33] (32 v-dims + ones
row for the softmax denominator Z), moving = e [128 j, 512 i]. Two
j-chunks run concurrently via col tiling (tile_position (0,0)/(0,64))
into partition bands [0:33] / [64:97] of one PSUM bank, accumulating
u[d,i] = sum_j v_aug[j,d] e[j,i] over all 32 j-chunks. Measured
~255ns/pair (= ~2 matmuls of 512 moving cols overlapped) -> PV ~33us.

  - i processed in two halves of 2048 (4 i-blocks of 512 each): 4 u
    accumulator banks + sim 2x[128,1024]x2bufs = 8 PSUM banks exactly.
  - loop is g-outer (j-group) within a half; PV emitted one group late
    so PE never waits on ACT exp (v5 trick).
  - epilogue per i-block: ACT copies u bands to SBUF bf16, two
    row-packed K=33 matmuls project u -> [65, 512] (w_out fold + Z in
    row 64, two PSUM tiles to avoid same-region accumulation races),
    DVE adds them into SBUF, DMA out.
  - host: out = out2[0:64]/out2[64] (+ b_out), like v5's host bias add.
  - q/k projections, l2-norm scales (folded into kp4), 4-strip row-
    packed sim and exp are v5 verbatim.

All matmul operands bf16 (f32r measures 4-10x slower than bf16 here).
PSUM rule: never two concurrent matmuls into column sub-ranges of one
bank (HW hang); partition sub-bands via col tiling are the documented
pattern and measure fine.
"""

import os
import sys

for _p in ("/opt/trn_rl_repo", "/root/.axon_site/_ro/trn_rl_repo"):
    if os.path.isdir(_p) and _p not in sys.path:
        sys.path.insert(0, _p)
        break

import numpy as np

import concourse.tile as tile
from concourse import bacc, mybir

F32 = mybir.dt.float32
BF16 = mybir.dt.bfloat16
N = 4096
C = 64
D = 32
SCALE = 10.0
N_CORES = 8
IB = 512          # i-block
NJ = N // 128     # 32 j-chunks
NG = 8            # j-groups of 4 chunks (one per strip)
NHB = 4           # i-blocks per half

REPEAT = int(os.environ.get("ATTN_REPEAT", "1"))


def build_nc(repeat=None):
    if repeat is None:
        repeat = REPEAT

    nc = bacc.Bacc(
        "TRN2",
        target_bir_lowering=False,
        debug=False,
        num_devices=N_CORES,
    )

    xb = nc.dram_tensor("xb", [C, N], F32, kind="ExternalInput").ap()
    wq = nc.dram_tensor("wq", [C, D], F32, kind="ExternalInput").ap()
    wk = nc.dram_tensor("wk", [C, D], F32, kind="ExternalInput").ap()
    wv = nc.dram_tensor("wv", [C, D], F32, kind="ExternalInput").ap()
    wo = nc.dram_tensor("wo", [D + 1, C + 1], F32, kind="ExternalInput").ap()
    out = nc.dram_tensor("out", [C + 1, N], F32, kind="ExternalOutput").ap()

    with tile.TileContext(nc) as tc:
        with (
            tc.tile_pool(name="consts", bufs=1) as consts,
            tc.tile_pool(name="persist", bufs=1) as persist,
            tc.tile_pool(name="esb", bufs=16) as esb,
            tc.tile_pool(name="epi", bufs=2) as epi,
            tc.tile_pool(name="sim_ps", bufs=2, space="PSUM") as sim_ps,
            tc.tile_pool(name="u_ps", bufs=4, space="PSUM") as u_ps_pool,
        ):
            wq_f = consts.tile([128, D], F32)
            wk_f = consts.tile([128, D], F32)
            wv_f = consts.tile([128, D], F32)
            woz_f = consts.tile([128, C + 1], F32)
            zero_b = consts.tile([128, 1], F32)
            for w_sb, w_dram in ((wq_f, wq), (wk_f, wk), (wv_f, wv)):
                nc.sync.dma_start(out=w_sb[0:C, :], in_=w_dram)
                nc.sync.dma_start(out=w_sb[C:128, :], in_=w_dram)
            nc.vector.memset(woz_f, 0.0)
            nc.sync.dma_start(out=woz_f[0 : D + 1, :], in_=wo)
            nc.sync.dma_start(out=woz_f[64 : 64 + D + 1, :], in_=wo)
            nc.vector.memset(zero_b, 0.0)
            wqr = consts.tile([128, D], BF16)
            wkr = consts.tile([128, D], BF16)
            wvr = consts.tile([128, D], BF16)
            woz = consts.tile([128, C + 1], BF16)
            nc.vector.tensor_copy(wqr, wq_f)
            nc.vector.tensor_copy(wkr, wk_f)
            nc.vector.tensor_copy(wvr, wv_f)
            nc.vector.tensor_copy(woz, woz_f)

            x_sb = consts.tile([128, N // 2], F32)
            xr_sb = consts.tile([128, N // 2], BF16)

            def x_ap(tile_, tok0, ntok):
                half, col = divmod(tok0, N // 2)
                p0 = half * C
                return tile_[p0 : p0 + C, col : col + ntok]

            def body():
                q4 = persist.tile([128, N], BF16)
                k4 = persist.tile([128, NJ, 128], BF16)
                sq_scr = persist.tile([128, 512], F32)
                ssqp_q = persist.tile([128, N // 512], F32)
                ssqp_k = persist.tile([128, N // 512], F32)
                # v_aug^T padded to 64 cols: [v (32) | ones (1) | zeros (31)]
                # so the PV col tiles write full 64-partition PSUM bands
                # (epilogue can then read the whole bank + use a single
                # K=128 projection matmul against the zero-padded woz).
                vt_sb = persist.tile([128, NJ, C], BF16)

                for t in range(N // 512):
                    nc.sync.dma_start(
                        out=x_ap(x_sb, t * 512, 512),
                        in_=xb[:, t * 512 : (t + 1) * 512],
                    )
                    nc.scalar.activation(
                        x_ap(xr_sb, t * 512, 512), x_ap(x_sb, t * 512, 512),
                        mybir.ActivationFunctionType.Copy, bias=0.0,
                    )

                row = lambda t: 0 if t < 4 else C
                for t in range(N // 512):
                    sl = slice(t * 512, (t + 1) * 512)
                    c0, c1 = t * 4, (t + 1) * 4
                    xa = x_ap(xr_sb, t * 512, 512)
                    w_q = wqr[row(t) : row(t) + C, :]
                    w_k = wkr[row(t) : row(t) + C, :]
                    ps_q = sim_ps.tile([128, 512], F32, tag="s3")
                    for s in range(4):
                        nc.tensor.matmul(
                            ps_q[32 * s : 32 * s + 32, :],
                            lhsT=w_q, rhs=xa, start=True, stop=True,
                            tile_position=(row(t), 32 * s),
                        )
                    nc.scalar.activation(
                        q4[:, sl], ps_q,
                        mybir.ActivationFunctionType.Copy, bias=0.0,
                    )
                    nc.scalar.activation(
                        sq_scr, ps_q, mybir.ActivationFunctionType.Square,
                        bias=zero_b, accum_out=ssqp_q[:, t : t + 1],
                    )
                    ps_k = sim_ps.tile([128, 512], F32, tag="s3")
                    for s in range(4):
                        nc.tensor.matmul(
                            ps_k[32 * s : 32 * s + 32, :],
                            lhsT=w_k, rhs=xa, start=True, stop=True,
                            tile_position=(row(t), 32 * s),
                        )
                    nc.scalar.activation(
                        k4[:, c0:c1, :], ps_k,
                        mybir.ActivationFunctionType.Copy, bias=0.0,
                    )
                    nc.scalar.activation(
                        sq_scr, ps_k, mybir.ActivationFunctionType.Square,
                        bias=zero_b, accum_out=ssqp_k[:, t : t + 1],
                    )

                # v^T per j-chunk: [128 j, 32 d] = (x chunk)^T @ wv.
                # Chunks jc and jc+16 sit in different partition halves of
                # xr_sb -> row-packed concurrent pairs.
                for jc in range(NJ // 2):
                    for cc, p0 in ((jc, 0), (jc + NJ // 2, C)):
                        ps_vt = sim_ps.tile(
                            [128, D], F32, tag="s3", name=f"ps_vt_{cc}"
                        )
                        nc.tensor.matmul(
                            ps_vt,
                            lhsT=x_ap(xr_sb, cc * 128, 128),
                            rhs=wvr[p0 : p0 + C, :],
                            start=True, stop=True,
                            tile_position=(p0, 0),
                        )
                        nc.scalar.activation(
                            vt_sb[:, cc, 0:D], ps_vt,
                            mybir.ActivationFunctionType.Copy, bias=0.0,
                        )
                nc.vector.memset(vt_sb[:, :, D : D + 1], 1.0)
                nc.vector.memset(vt_sb[:, :, D + 1 : C], 0.0)

                ssq_q = persist.tile([128, 1], F32)
                ssq_k = persist.tile([128, 1], F32)
                nc.vector.reduce_sum(
                    out=ssq_q, in_=ssqp_q, axis=mybir.AxisListType.X
                )
                nc.vector.reduce_sum(
                    out=ssq_k, in_=ssqp_k, axis=mybir.AxisListType.X
                )
                lq = persist.tile([128, 1], F32)
                lk = persist.tile([128, 1], F32)
                nc.scalar.activation(
                    lq, ssq_q, mybir.ActivationFunctionType.Ln,
                    bias=zero_b, scale=1.0 / (SCALE * SCALE),
                )
                nc.scalar.activation(
                    lk, ssq_k, mybir.ActivationFunctionType.Ln,
                    bias=zero_b,
                )
                nc.vector.tensor_add(lq, lq, lk)
                cscale = persist.tile([128, 1], F32)
                nc.scalar.activation(
                    cscale, lq, mybir.ActivationFunctionType.Exp,
                    bias=zero_b, scale=-0.5,
                )

                kp4 = persist.tile([128, NJ // 4, 128], BF16)
                for lo, hi in ((0, 1), (1, NJ // 4)):
                    for s in range(4):
                        nc.vector.tensor_scalar_mul(
                            kp4[32 * s : 32 * s + 32, lo:hi, :],
                            k4[
                                32 * s : 32 * s + 32,
                                s + 4 * lo : s + 4 * (hi - 1) + 1 : 4,
                                :,
                            ],
                            cscale[32 * s : 32 * s + 32],
                        )

                # ---- main loop ----
                u_tiles = {}
                pend = None

                def emit_pv(half, g, e_tiles):
                    for ib in range(NHB):
                        for h in range(2):
                            e_t = e_tiles[(ib, h)]
                            st = g == 0 and h == 0
                            sp = g == NG - 1 and h == 1
                            u_t = u_tiles[(half, ib)]
                            nc.tensor.matmul(
                                u_t[0:64, :],
                                lhsT=vt_sb[:, 4 * g + 2 * h, :],
                                rhs=e_t[:, 0:IB],
                                start=st, stop=sp,
                                tile_position=(0, 0),
                            )
                            nc.tensor.matmul(
                                u_t[64:128, :],
                                lhsT=vt_sb[:, 4 * g + 2 * h + 1, :],
                                rhs=e_t[:, IB : 2 * IB],
                                start=st, stop=sp,
                                tile_position=(0, 64),
                            )

                def emit_epilogue(half):
                    for ib in range(NHB):
                        u_t = u_tiles.pop((half, ib))
                        u_sb = epi.tile([128, IB], BF16, tag="usb")
                        nc.scalar.activation(
                            u_sb, u_t,
                            mybir.ActivationFunctionType.Copy, bias=0.0,
                        )
                        o2 = sim_ps.tile(
                            [C + 1, IB], F32, tag="s3", name=f"o2_{half}_{ib}"
                        )
                        nc.tensor.matmul(
                            o2, lhsT=woz, rhs=u_sb,
                            start=True, stop=True, tile_position=(0, 0),
                        )
                        o_sb = epi.tile([C + 1, IB], F32, tag="osb")
                        nc.scalar.activation(
                            o_sb, o2,
                            mybir.ActivationFunctionType.Copy, bias=0.0,
                        )
                        i0 = half * (N // 2) + ib * IB
                        nc.sync.dma_start(
                            out=out[:, i0 : i0 + IB], in_=o_sb
                        )

                for half in range(2):
                    for ib in range(NHB):
                        u_tiles[(half, ib)] = u_ps_pool.tile(
                            [128, IB], F32, tag="u",
                            name=f"u_ps_{half}_{ib}",
                        )
                    for g in range(NG):
                        e_tiles = {}
                        for ib in range(NHB):
                            i0 = half * (N // 2) + ib * IB
                            isl = slice(i0, i0 + IB)
                            for h in range(2):
                                s_ps = sim_ps.tile(
                                    [128, 2 * IB], F32, tag="s3",
                                    name=f"s_ps_{half}_{g}_{ib}_{h}",
                                )
                                for j in range(2):
                                    strip = 2 * h + j
                                    nc.tensor.matmul(
                                        s_ps[:, j * IB : (j + 1) * IB],
                                        lhsT=kp4[
                                            32 * strip : 32 * strip + 32,
                                            g, :,
                                        ],
                                        rhs=q4[
                                            32 * strip : 32 * strip + 32, isl
                                        ],
                                        start=True, stop=True,
                                        tile_position=(32 * strip, 0),
                                    )
                                e_t = esb.tile(
                                    [128, 2 * IB], BF16, tag="e2",
                                    name=f"e2_{half}_{g}_{ib}_{h}",
                                )
                                nc.scalar.activation(
                                    e_t, s_ps,
                                    mybir.ActivationFunctionType.Exp,
                                    bias=zero_b,
                                )
                                e_tiles[(ib, h)] = e_t
                        if pend is not None:
                            ph, pg, pe = pend
                            emit_pv(ph, pg, pe)
                            if pg == NG - 1:
                                emit_epilogue(ph)
                        pend = (half, g, e_tiles)
                ph, pg, pe = pend
                emit_pv(ph, pg, pe)
                emit_epilogue(ph)

            for _rep in range(repeat):
                body()

    nc.compile()
    return nc


_NC_CACHE = {}


def _get_nc():
    key = REPEAT
    if key not in _NC_CACHE:
        _NC_CACHE[key] = build_nc()
    return _NC_CACHE[key]


def _make_in_maps(x, w_qkv, w_out):
    b, c, X, Y, Z = x.shape
    xr = np.ascontiguousarray(x.reshape(b, c, X * Y * Z), dtype=np.float32)
    w_qkv = np.asarray(w_qkv, dtype=np.float32)
    w_out = np.asarray(w_out, dtype=np.float32)
    in_maps = []
    for core in range(N_CORES):
        bi, h = divmod(core, 4)
        hs = slice(h * D, (h + 1) * D)
        wo_aug = np.zeros((D + 1, C + 1), dtype=np.float32)
        wo_aug[0:D, 0:C] = w_out[:, hs].T
        wo_aug[D, C] = 1.0
        in_maps.append(
            {
                "xb": xr[bi],
                "wq": np.ascontiguousarray(w_qkv[hs, :].T),
                "wk": np.ascontiguousarray(w_qkv[128 + h * D : 128 + (h + 1) * D, :].T),
                "wv": np.ascontiguousarray(w_qkv[256 + h * D : 256 + (h + 1) * D, :].T),
                "wo": wo_aug,
            }
        )
    return in_maps


def _gather(results, x_shape, b_out):
    b, c, X, Y, Z = x_shape
    n = X * Y * Z
    out = np.zeros((b, c, n), dtype=np.float32)
    for core in range(N_CORES):
        bi = core // 4
        r = results[core]["out"]
        out[bi] += r[0:C] / r[C : C + 1]
    out += np.asarray(b_out, dtype=np.float32)[None, :, None]
    return out.reshape(b, c, X, Y, Z)


def kernel(x, w_qkv, w_out, b_out):
    from concourse.bass_utils import run_bass_kernel_spmd

    x = np.asarray(x)
    nc = _get_nc()
    in_maps = _make_in_maps(x, w_qkv, w_out)
    try:
        res = run_bass_kernel_spmd(nc, in_maps, list(range(N_CORES))).results
    except Exception:
        # A wedged NeuronCore (e.g. NRT_EXEC_UNIT_UNRECOVERABLE left over
        # from an earlier crashed process) usually recovers on re-run.
        res = run_bass_kernel_spmd(nc, in_maps, list(range(N_CORES))).results
    return _gather(res, x.shape, b_out)
# ---- appended runner/benchmark helpers (used by test.py, not the harness) ----


def _make_runner(nc, in_maps):
    """Build a reusable jitted 8-core runner with device-resident inputs.

    Mirrors bass2jax.run_bass_via_pjrt's multi-core tail, minus output
    donation, so repeated timed calls reuse on-device buffers.
    """
    import jax
    from jax.experimental.shard_map import shard_map
    from jax.sharding import Mesh, PartitionSpec

    from concourse import bass2jax, mybir as _mybir

    bass2jax.install_neuronx_cc_hook()

    partition_name = (
        nc.partition_id_tensor.name if nc.partition_id_tensor else None
    )
    in_names, out_names, out_avals, zero_outs = [], [], [], []
    for alloc in nc.m.functions[0].allocations:
        if not isinstance(alloc, _mybir.MemoryLocationSet):
            continue
        name = alloc.memorylocations[0].name
        if alloc.kind == "ExternalInput":
            if name != partition_name:
                in_names.append(name)
        elif alloc.kind == "ExternalOutput":
            out_names.append(name)
            shape = tuple(alloc.tensor_shape)
            dtype = _mybir.dt.np(alloc.dtype)
            out_avals.append(jax.core.ShapedArray(shape, dtype))
            zero_outs.append(np.zeros(shape, dtype))
    n_params = len(in_names)
    all_in_names = in_names + out_names
    if partition_name is not None:
        all_in_names = all_in_names + [partition_name]

    def _body(*args):
        operands = list(args)
        if partition_name is not None:
            operands.append(bass2jax.partition_id_tensor())
        outs = bass2jax._bass_exec_p.bind(
            *operands,
            out_avals=tuple(out_avals),
            in_names=tuple(all_in_names),
            out_names=tuple(out_names),
            lowering_input_output_aliases=(),
            sim_require_finite=True,
            sim_require_nnan=True,
            nc=nc,
        )
        return tuple(outs)

    devices = jax.devices()[:N_CORES]
    mesh = Mesh(np.asarray(devices), ("core",))
    n_outs = len(out_names)
    sharded = jax.jit(
        shard_map(
            _body,
            mesh=mesh,
            in_specs=(PartitionSpec("core"),) * (n_params + n_outs),
            out_specs=(PartitionSpec("core"),) * n_outs,
            check_rep=False,
        ),
        keep_unused=True,
    )
    sharding = jax.sharding.NamedSharding(mesh, PartitionSpec("core"))
    concat_in = [
        jax.device_put(
            np.concatenate([np.asarray(m[name]) for m in in_maps], axis=0),
            sharding,
        )
        for name in in_names
    ]
    concat_zeros = [
        jax.device_put(
            np.zeros((N_CORES * z.shape[0], *z.shape[1:]), z.dtype), sharding
        )
        for z in zero_outs
    ]

    def run():
        return sharded(*concat_in, *concat_zeros)

    return run


# revision 8
# speedup vs baseline: 2.2015x; 2.2015x over previous
"""Trainium2 Bass kernel v6: v-stationary col-packed PV in u-space.

v5 bottleneck: PV ran e-stationary (lhsT = exp(S^T) [128j x 128i] chunk,
rhs = vw [128, 65]) -> 1024 matmuls, each LoadStationary-bound (~80ns
measured: LS of a 128-col stationary ~53ns > 27ns of moving stream) ->
~82us of PE time on PV alone.

v6 flips PV: stationary = v_aug^T chunk [128 j, 33] (32 v-dims + ones
row for the softmax denominator Z), moving = e [128 j, 512 i]. Two
j-chunks run concurrently via col tiling (tile_position (0,0)/(0,64))
into partition bands [0:33] / [64:97] of one PSUM bank, accumulating
u[d,i] = sum_j v_aug[j,d] e[j,i] over all 32 j-chunks. Measured
~255ns/pair (= ~2 matmuls of 512 moving cols overlapped) -> PV ~33us.

  - i processed in two halves of 2048 (4 i-blocks of 512 each): 4 u
    accumulator banks + sim 2x[128,1024]x2bufs = 8 PSUM banks exactly.
  - loop is g-outer (j-group) within a half; PV emitted one group late
    so PE never waits on ACT exp (v5 trick).
  - epilogue per i-block: ACT copies u bands to SBUF bf16, two
    row-packed K=33 matmuls project u -> [65, 512] (w_out fold + Z in
    row 64, two PSUM tiles to avoid same-region accumulation races),
    DVE adds them into SBUF, DMA out.
  - host: out = out2[0:64]/out2[64] (+ b_out), like v5's host bias add.
  - q/k projections, l2-norm scales (folded into kp4), 4-strip row-
    packed sim and exp are v5 verbatim.

All matmul operands bf16 (f32r measures 4-10x slower than bf16 here).
PSUM rule: never two concurrent matmuls into column sub-ranges of one
bank (HW hang); partition sub-bands via col tiling are the documented
pattern and measure fine.
"""

import os
import sys

for _p in ("/opt/trn_rl_repo", "/root/.axon_site/_ro/trn_rl_repo"):
    if os.path.isdir(_p) and _p not in sys.path:
        sys.path.insert(0, _p)
        break

import numpy as np

import concourse.tile as tile
from concourse import bacc, mybir

F32 = mybir.dt.float32
BF16 = mybir.dt.bfloat16
N = 4096
C = 64
D = 32
SCALE = 10.0
N_CORES = 8
IB = 512          # i-block
NJ = N // 128     # 32 j-chunks
NG = 8            # j-groups of 4 chunks (one per strip)
NHB = 4           # i-blocks per half

REPEAT = int(os.environ.get("ATTN_REPEAT", "1"))


def build_nc(repeat=None):
    if repeat is None:
        repeat = REPEAT

    nc = bacc.Bacc(
        "TRN2",
        target_bir_lowering=False,
        debug=False,
        num_devices=N_CORES,
    )

    xb = nc.dram_tensor("xb", [C, N], F32, kind="ExternalInput").ap()
    wq = nc.dram_tensor("wq", [C, D], F32, kind="ExternalInput").ap()
    wk = nc.dram_tensor("wk", [C, D], F32, kind="ExternalInput").ap()
    wv = nc.dram_tensor("wv", [C, D], F32, kind="ExternalInput").ap()
    wo = nc.dram_tensor("wo", [D + 1, C + 1], F32, kind="ExternalInput").ap()
    out = nc.dram_tensor("out", [C + 1, N], F32, kind="ExternalOutput").ap()

    with tile.TileContext(nc) as tc:
        with (
            tc.tile_pool(name="consts", bufs=1) as consts,
            tc.tile_pool(name="persist", bufs=1) as persist,
            tc.tile_pool(name="esb", bufs=16) as esb,
            tc.tile_pool(name="epi", bufs=2) as epi,
            tc.tile_pool(name="sim_ps", bufs=2, space="PSUM") as sim_ps,
            tc.tile_pool(name="u_ps", bufs=4, space="PSUM") as u_ps_pool,
        ):
            wq_f = consts.tile([128, D], F32)
            wk_f = consts.tile([128, D], F32)
            wv_f = consts.tile([128, D], F32)
            woz_f = consts.tile([128, C + 1], F32)
            zero_b = consts.tile([128, 1], F32)
            for w_sb, w_dram in ((wq_f, wq), (wk_f, wk), (wv_f, wv)):
                nc.sync.dma_start(out=w_sb[0:C, :], in_=w_dram)
                nc.sync.dma_start(out=w_sb[C:128, :], in_=w_dram)
            nc.vector.memset(woz_f, 0.0)
            nc.sync.dma_start(out=woz_f[0 : D + 1, :], in_=wo)
            nc.sync.dma_start(out=woz_f[64 : 64 + D + 1, :], in_=wo)
            nc.vector.memset(zero_b, 0.0)
            wqr = consts.tile([128, D], BF16)
            wkr = consts.tile([128, D], BF16)
            wvr = consts.tile([128, D], BF16)
            woz = consts.tile([128, C + 1], BF16)
            nc.vector.tensor_copy(wqr, wq_f)
            nc.vector.tensor_copy(wkr, wk_f)
            nc.vector.tensor_copy(wvr, wv_f)
            nc.vector.tensor_copy(woz, woz_f)

            x_sb = consts.tile([128, N // 2], F32)
            xr_sb = consts.tile([128, N // 2], BF16)

            def x_ap(tile_, tok0, ntok):
                half, col = divmod(tok0, N // 2)
                p0 = half * C
                return tile_[p0 : p0 + C, col : col + ntok]

            def body():
                q4 = persist.tile([128, N], BF16)
                k4 = persist.tile([128, NJ, 128], BF16)
                sq_scr = persist.tile([128, 512], F32)
                ssqp_q = persist.tile([128, N // 512], F32)
                ssqp_k = persist.tile([128, N // 512], F32)
                # v_aug^T padded to 64 cols: [v (32) | ones (1) | zeros (31)]
                # so the PV col tiles write full 64-partition PSUM bands
                # (epilogue can then read the whole bank + use a single
                # K=128 projection matmul against the zero-padded woz).
                vt_sb = persist.tile([128, NJ, C], BF16)

                for t in range(N // 512):
                    nc.sync.dma_start(
                        out=x_ap(x_sb, t * 512, 512),
                        in_=xb[:, t * 512 : (t + 1) * 512],
                    )
                    nc.scalar.activation(
                        x_ap(xr_sb, t * 512, 512), x_ap(x_sb, t * 512, 512),
                        mybir.ActivationFunctionType.Copy, bias=0.0,
                    )

                row = lambda t: 0 if t < 4 else C
                for t in range(N // 512):
                    sl = slice(t * 512, (t + 1) * 512)
                    c0, c1 = t * 4, (t + 1) * 4
                    xa = x_ap(xr_sb, t * 512, 512)
                    w_q = wqr[row(t) : row(t) + C, :]
                    w_k = wkr[row(t) : row(t) + C, :]
                    ps_q = sim_ps.tile([128, 512], F32, tag="s3")
                    for s in range(4):
                        nc.tensor.matmul(
                            ps_q[32 * s : 32 * s + 32, :],
                            lhsT=w_q, rhs=xa, start=True, stop=True,
                            tile_position=(row(t), 32 * s),
                        )
                    nc.scalar.activation(
                        q4[:, sl], ps_q,
                        mybir.ActivationFunctionType.Copy, bias=0.0,
                    )
                    nc.scalar.activation(
                        sq_scr, ps_q, mybir.ActivationFunctionType.Square,
                        bias=zero_b, accum_out=ssqp_q[:, t : t + 1],
                    )
                    ps_k = sim_ps.tile([128, 512], F32, tag="s3")
                    for s in range(4):
                        nc.tensor.matmul(
                            ps_k[32 * s : 32 * s + 32, :],
                            lhsT=w_k, rhs=xa, start=True, stop=True,
                            tile_position=(row(t), 32 * s),
                        )
                    nc.scalar.activation(
                        k4[:, c0:c1, :], ps_k,
                        mybir.ActivationFunctionType.Copy, bias=0.0,
                    )
                    nc.scalar.activation(
                        sq_scr, ps_k, mybir.ActivationFunctionType.Square,
                        bias=zero_b, accum_out=ssqp_k[:, t : t + 1],
                    )

                # v^T per j-chunk: [128 j, 32 d] = (x chunk)^T @ wv.
                # Chunks jc and jc+16 sit in different partition halves of
                # xr_sb -> row-packed concurrent pairs.
                for jc in range(NJ // 2):
                    for cc, p0 in ((jc, 0), (jc + NJ // 2, C)):
                        ps_vt = sim_ps.tile(
                            [128, D], F32, tag="s3", name=f"ps_vt_{cc}"
                        )
                        nc.tensor.matmul(
                            ps_vt,
                            lhsT=x_ap(xr_sb, cc * 128, 128),
                            rhs=wvr[p0 : p0 + C, :],
                            start=True, stop=True,
                            tile_position=(p0, 0),
                        )
                        nc.vector.tensor_copy(vt_sb[:, cc, 0:D], ps_vt)
                nc.vector.memset(vt_sb[:, :, D : D + 1], 1.0)
                nc.vector.memset(vt_sb[:, :, D + 1 : C], 0.0)

                ssq_q = persist.tile([128, 1], F32)
                ssq_k = persist.tile([128, 1], F32)
                nc.vector.reduce_sum(
                    out=ssq_q, in_=ssqp_q, axis=mybir.AxisListType.X
                )
                nc.vector.reduce_sum(
                    out=ssq_k, in_=ssqp_k, axis=mybir.AxisListType.X
                )
                lq = persist.tile([128, 1], F32)
                lk = persist.tile([128, 1], F32)
                nc.scalar.activation(
                    lq, ssq_q, mybir.ActivationFunctionType.Ln,
                    bias=zero_b, scale=1.0 / (SCALE * SCALE),
                )
                nc.scalar.activation(
                    lk, ssq_k, mybir.ActivationFunctionType.Ln,
                    bias=zero_b,
                )
                nc.vector.tensor_add(lq, lq, lk)
                cscale = persist.tile([128, 1], F32)
                nc.scalar.activation(
                    cscale, lq, mybir.ActivationFunctionType.Exp,
                    bias=zero_b, scale=-0.5,
                )

                kp4 = persist.tile([128, NJ // 4, 128], BF16)
                for lo, hi in ((0, 1), (1, NJ // 4)):
                    for s in range(4):
                        nc.vector.tensor_scalar_mul(
                            kp4[32 * s : 32 * s + 32, lo:hi, :],
                            k4[
                                32 * s : 32 * s + 32,
                                s + 4 * lo : s + 4 * (hi - 1) + 1 : 4,
                                :,
                            ],
                            cscale[32 * s : 32 * s + 32],
                        )

                # ---- main loop ----
                u_tiles = {}
                pend = None

                def emit_pv(half, g, ib, e_pair):
                    u_t = u_tiles[(half, ib)]
                    for h in range(2):
                        e_t = e_pair[h]
                        st = g == 0 and h == 0
                        sp = g == NG - 1 and h == 1
                        nc.tensor.matmul(
                            u_t[0:64, :],
                            lhsT=vt_sb[:, 4 * g + 2 * h, :],
                            rhs=e_t[:, 0:IB],
                            start=st, stop=sp,
                            tile_position=(0, 0),
                        )
                        nc.tensor.matmul(
                            u_t[64:128, :],
                            lhsT=vt_sb[:, 4 * g + 2 * h + 1, :],
                            rhs=e_t[:, IB : 2 * IB],
                            start=st, stop=sp,
                            tile_position=(0, 64),
                        )

                def emit_epilogue(half, ib):
                    u_t = u_tiles.pop((half, ib))
                    u_sb = epi.tile([128, IB], BF16, tag="usb")
                    nc.scalar.activation(
                        u_sb, u_t,
                        mybir.ActivationFunctionType.Copy, bias=0.0,
                    )
                    o2 = sim_ps.tile(
                        [C + 1, IB], F32, tag="s3", name=f"o2_{half}_{ib}"
                    )
                    nc.tensor.matmul(
                        o2, lhsT=woz, rhs=u_sb,
                        start=True, stop=True, tile_position=(0, 0),
                    )
                    o_sb = epi.tile([C + 1, IB], F32, tag="osb")
                    nc.scalar.activation(
                        o_sb, o2,
                        mybir.ActivationFunctionType.Copy, bias=0.0,
                    )
                    i0 = half * (N // 2) + ib * IB
                    nc.sync.dma_start(out=out[:, i0 : i0 + IB], in_=o_sb)

                for half in range(2):
                    for ib in range(NHB):
                        u_tiles[(half, ib)] = u_ps_pool.tile(
                            [128, IB], F32, tag="u",
                            name=f"u_ps_{half}_{ib}",
                        )
                    for g in range(NG):
                        for ib in range(NHB):
                            i0 = half * (N // 2) + ib * IB
                            isl = slice(i0, i0 + IB)
                            e_pair = {}
                            for h in range(2):
                                s_ps = sim_ps.tile(
                                    [128, 2 * IB], F32, tag="s3",
                                    name=f"s_ps_{half}_{g}_{ib}_{h}",
                                )
                                for j in range(2):
                                    strip = 2 * h + j
                                    nc.tensor.matmul(
                                        s_ps[:, j * IB : (j + 1) * IB],
                                        lhsT=kp4[
                                            32 * strip : 32 * strip + 32,
                                            g, :,
                                        ],
                                        rhs=q4[
                                            32 * strip : 32 * strip + 32, isl
                                        ],
                                        start=True, stop=True,
                                        tile_position=(32 * strip, 0),
                                    )
                                e_t = esb.tile(
                                    [128, 2 * IB], BF16, tag="e2",
                                    name=f"e2_{half}_{g}_{ib}_{h}",
                                )
                                nc.scalar.activation(
                                    e_t, s_ps,
                                    mybir.ActivationFunctionType.Exp,
                                    bias=zero_b,
                                )
                                e_pair[h] = e_t
                            if pend is not None:
                                ph, pg, pib, pe = pend
                                emit_pv(ph, pg, pib, pe)
                                if pg == NG - 1:
                                    emit_epilogue(ph, pib)
                            pend = (half, g, ib, e_pair)
                ph, pg, pib, pe = pend
                emit_pv(ph, pg, pib, pe)
                emit_epilogue(ph, pib)

            for _rep in range(repeat):
                body()

    nc.compile()
    return nc


_NC_CACHE = {}


def _get_nc():
    key = REPEAT
    if key not in _NC_CACHE:
        _NC_CACHE[key] = build_nc()
    return _NC_CACHE[key]


def _make_in_maps(x, w_qkv, w_out):
    b, c, X, Y, Z = x.shape
    xr = np.ascontiguousarray(x.reshape(b, c, X * Y * Z), dtype=np.float32)
    w_qkv = np.asarray(w_qkv, dtype=np.float32)
    w_out = np.asarray(w_out, dtype=np.float32)
    in_maps = []
    for core in range(N_CORES):
        bi, h = divmod(core, 4)
        hs = slice(h * D, (h + 1) * D)
        wo_aug = np.zeros((D + 1, C + 1), dtype=np.float32)
        wo_aug[0:D, 0:C] = w_out[:, hs].T
        wo_aug[D, C] = 1.0
        in_maps.append(
            {
                "xb": xr[bi],
                "wq": np.ascontiguousarray(w_qkv[hs, :].T),
                "wk": np.ascontiguousarray(w_qkv[128 + h * D : 128 + (h + 1) * D, :].T),
                "wv": np.ascontiguousarray(w_qkv[256 + h * D : 256 + (h + 1) * D, :].T),
                "wo": wo_aug,
            }
        )
    return in_maps


def _gather(results, x_shape, b_out):
    b, c, X, Y, Z = x_shape
    n = X * Y * Z
    out = np.zeros((b, c, n), dtype=np.float32)
    for core in range(N_CORES):
        bi = core // 4
        r = results[core]["out"]
        out[bi] += r[0:C] / r[C : C + 1]
    out += np.asarray(b_out, dtype=np.float32)[None, :, None]
    return out.reshape(b, c, X, Y, Z)


def kernel(x, w_qkv, w_out, b_out):
    from concourse.bass_utils import run_bass_kernel_spmd

    x = np.asarray(x)
    nc = _get_nc()
    in_maps = _make_in_maps(x, w_qkv, w_out)
    try:
        res = run_bass_kernel_spmd(nc, in_maps, list(range(N_CORES))).results
    except Exception:
        # A wedged NeuronCore (e.g. NRT_EXEC_UNIT_UNRECOVERABLE left over
        # from an earlier crashed process) usually recovers on re-run.
        res = run_bass_kernel_spmd(nc, in_maps, list(range(N_CORES))).results
    return _gather(res, x.shape, b_out)
# ---- appended runner/benchmark helpers (used by test.py, not the harness) ----


def _make_runner(nc, in_maps):
    """Build a reusable jitted 8-core runner with device-resident inputs.

    Mirrors bass2jax.run_bass_via_pjrt's multi-core tail, minus output
    donation, so repeated timed calls reuse on-device buffers.
    """
    import jax
    from jax.experimental.shard_map import shard_map
    from jax.sharding import Mesh, PartitionSpec

    from concourse import bass2jax, mybir as _mybir

    bass2jax.install_neuronx_cc_hook()

    partition_name = (
        nc.partition_id_tensor.name if nc.partition_id_tensor else None
    )
    in_names, out_names, out_avals, zero_outs = [], [], [], []
    for alloc in nc.m.functions[0].allocations:
        if not isinstance(alloc, _mybir.MemoryLocationSet):
            continue
        name = alloc.memorylocations[0].name
        if alloc.kind == "ExternalInput":
            if name != partition_name:
                in_names.append(name)
        elif alloc.kind == "ExternalOutput":
            out_names.append(name)
            shape = tuple(alloc.tensor_shape)
            dtype = _mybir.dt.np(alloc.dtype)
            out_avals.append(jax.core.ShapedArray(shape, dtype))
            zero_outs.append(np.zeros(shape, dtype))
    n_params = len(in_names)
    all_in_names = in_names + out_names
    if partition_name is not None:
        all_in_names = all_in_names + [partition_name]

    def _body(*args):
        operands = list(args)
        if partition_name is not None:
            operands.append(bass2jax.partition_id_tensor())
        outs = bass2jax._bass_exec_p.bind(
            *operands,
            out_avals=tuple(out_avals),
            in_names=tuple(all_in_names),
            out_names=tuple(out_names),
            lowering_input_output_aliases=(),
            sim_require_finite=True,
            sim_require_nnan=True,
            nc=nc,
        )
        return tuple(outs)

    devices = jax.devices()[:N_CORES]
    mesh = Mesh(np.asarray(devices), ("core",))
    n_outs = len(out_names)
    sharded = jax.jit(
        shard_map(
            _body,
            mesh=mesh,
            in_specs=(PartitionSpec("core"),) * (n_params + n_outs),
            out_specs=(PartitionSpec("core"),) * n_outs,
            check_rep=False,
        ),
        keep_unused=True,
    )
    sharding = jax.sharding.NamedSharding(mesh, PartitionSpec("core"))
    concat_in = [
        jax.device_put(
            np.concatenate([np.asarray(m[name]) for m in in_maps], axis=0),
            sharding,
        )
        for name in in_names
    ]
    concat_zeros = [
        jax.device_put(
            np.zeros((N_CORES * z.shape[0], *z.shape[1:]), z.dtype), sharding
        )
        for z in zero_outs
    ]

    def run():
        return sharded(*concat_in, *concat_zeros)

    return run
